# revision 78
# baseline (speedup 1.0000x reference)
"""Trainium2 Bass kernel for nn_ALayer_DR1_wh_light_v1 (dense_cnn).

Data-parallel over batch: 16 samples per NeuronCore, no collectives.

v3 schedule: v2 conv/attention interleave plus fp8 DoubleRow for taps
(4, 0, 8) — error measured 1.79e-2 against the 2e-2 gate, the 3-tap
subset with the best margin/savings ratio.  fp8 matmuls are w-trimmed
(boundary columns skipped; zero-padded rows contribute nothing).  The
final stores are split across both HWDGE queues.
"""
import numpy as np
import ml_dtypes

import concourse.bass as bass
import concourse.mybir as mybir
import concourse.tile as tile
from concourse.bass_utils import run_bass_kernel_spmd
from concourse.vector_clock import ScopedClock

F32 = mybir.dt.float32
BF16 = mybir.dt.bfloat16
FP8 = mybir.dt.float8e4
AX = mybir.AxisListType.X
AF = mybir.ActivationFunctionType
DR = mybir.MatmulPerfMode.DoubleRow

B, C, H, W = 128, 1024, 8, 8
L = H * W
NCORES = 8
BS = B // NCORES      # 16 samples per core
G = 2                 # sample groups
GB = BS // G          # 8 samples per group
NG = GB * L           # 512 = matmul N per group
NCT = 8               # channel tiles
NOT = 8               # output channel tiles

# ---- precision config ----------------------------------------------------
FP8_TAPS = (4, 1, 5)  # taps (j) computed in fp8 DoubleRow; () = all bf16
W1_FP8 = False        # spatial-attention w1 matmul in fp8 DoubleRow

XS = 8.0              # x prescale (host)
WS = 64.0             # conv weight prescale (host); psum scale = XS*WS

BF_TAPS = [j for j in (4, 1, 2, 3, 5, 6, 7, 0, 8) if j not in FP8_TAPS]
NBF = len(BF_TAPS)
SCHED_BF = [(j, ct) for j in BF_TAPS for ct in range(NCT)]   # bf16 steps
FP8_PAIRS = [(j, q) for j in FP8_TAPS for q in range(NCT // 2)]
NSTEP = len(SCHED_BF) + len(FP8_PAIRS)
PER_BANK = NSTEP      # matmul emissions per psum bank per group

# catch-up thresholds (tuned from traces)
K1 = 29               # step at which ot6 joins (bank freed by psz/psa)
K2 = 31               # step at which ot7 joins (bank freed by psf/psb)
NTAIL = 8             # trailing steps emitted bank-major so obs overlap

# psf chunk interleave order (mq = kt//4), deadline-ordered for SCHED_BF
MQ_ORDER = [2 * BF_TAPS[0], 2 * BF_TAPS[0] + 1]
for j in BF_TAPS[1:]:
    MQ_ORDER += [2 * j, 2 * j + 1]
for j in FP8_TAPS:
    MQ_ORDER += [2 * j, 2 * j + 1]

LOOKAHEAD = 8         # wsl DMA issue lookahead (steps)

TRACE = False
TRACE_DIR = None
LAST_PROFILE = {}

# ---------------------------------------------------------------------------
# Workaround: the staged walrus rejects instructions with >1 sync-wait
# command. Hoist excess waits onto same-engine NOPs inserted before the
# instruction (engine queues issue in order, so semantics are unchanged).
_MAXW = 1
_ctr = [0]


def _split_excess_waits(nc):
    for f in nc.m.functions:
        for blk in f.blocks:
            insts = blk.instructions
            new = []
            changed = False
            for inst in insts:
                si = inst.sync_info
                waits = list(si.on_wait) if si and si.on_wait else []
                if len(waits) > _MAXW:
                    excess = waits[:-_MAXW]
                    si.on_wait = waits[-_MAXW:]
                    while excess:
                        chunk = excess[:_MAXW]
                        excess = excess[_MAXW:]
                        _ctr[0] += 1
                        new.append(mybir.InstNoOp(
                            name=f"I-wsplit-{_ctr[0]}",
                            engine=inst.engine,
                            sync_info=mybir.SyncInfo(on_wait=chunk, on_update=[]),
                        ))
                    changed = True
                new.append(inst)
            if changed:
                insts.clear()
                insts.extend(new)


class _TileContext(tile.TileContext):
    def _drain_and_barrier(self, tick_clock, wait_clock):
        drain_inst = self.nc.sync.drain()
        wait_clock.add_sem_waits(
            drain_inst.ins, ScopedClock({None: tick_clock.global_clock})
        )
        self.nc.all_engine_barrier()
        assert self.sems is not None
        popped = self.nc._tile_sem_poison_stack.pop()
        assert popped is self._sem_poison
        self.nc.clear_and_free_semaphores(list(self.sems.allocated().values()))
        self.nc.all_engine_barrier()


# ---------------------------------------------------------------------------

def _tap_geom(j):
    dh, dw = j // 3, j % 3
    h0 = 1 if dh == 0 else 0
    nh = 8 - (1 if dh != 1 else 0)
    w0 = 1 if dw == 0 else 0
    nw = 8 - (1 if dw != 1 else 0)
    return dh, dw, h0, nh, w0, nw


def _build():
    nc = bass.Bass()
    xsp = nc.declare_dram_parameter("xsp", [C, BS * 100], BF16, isOutput=False)
    wttb = nc.declare_dram_parameter("wttb", [NBF * NCT, 128, 1024], BF16,
                                     isOutput=False)
    w1d = nc.declare_dram_parameter(
        "w1d", [C, 8 * 512], FP8 if W1_FP8 else BF16, isOutput=False)
    w67d = nc.declare_dram_parameter("w67d", [128, K2 * 256], BF16,
                                     isOutput=False)
    f1t = nc.declare_dram_parameter("f1t", [128, 8 * 64], BF16, isOutput=False)
    f2pt = nc.declare_dram_parameter("f2pt", [64, 9216], BF16, isOutput=False)
    w2t = nc.declare_dram_parameter("w2t", [128, 4 * 8], BF16, isOutput=False)
    identd = nc.declare_dram_parameter("identd", [32, 32], BF16, isOutput=False)
    if FP8_PAIRS:
        wtt8 = nc.declare_dram_parameter(
            "wtt8", [len(FP8_PAIRS), 128, 2048], FP8, isOutput=False)
    out = nc.declare_dram_parameter("out", [NOT, 128, BS * L], F32, isOutput=True)

    with _TileContext(nc) as tc:
        with (
            tc.tile_pool(name="pfix", bufs=1) as pfix,
            tc.tile_pool(name="pxpad", bufs=1) as pxpad,
            tc.tile_pool(name="pmdp", bufs=1) as pmdp,
            tc.tile_pool(name="pmd", bufs=14) as pmd,
            tc.tile_pool(name="pw", bufs=8) as pw,
            tc.tile_pool(name="pw8", bufs=4) as pw8,
            tc.tile_pool(name="pwm", bufs=8) as pwm,
            tc.tile_pool(name="pw1", bufs=3) as pw1,
            tc.tile_pool(name="pout", bufs=4) as pout,
            tc.tile_pool(name="ps", bufs=1, space="PSUM") as ps,
        ):
            # ---- persistent small tensors
            ybar = pfix.tile([128, NCT * BS], BF16)        # (ct, b)
            ys = pfix.tile([128, 9 * NCT * BS], BF16)      # (kt, b), kt=j*8+ct
            f1sb = pfix.tile([128, NCT * 64], BF16)
            f2sb = pfix.tile([64, 9216], BF16)
            w2sb = pfix.tile([128, 4 * 8], BF16)
            identsb = pfix.tile([32, 32], BF16)
            t1 = pfix.tile([64, BS], BF16)
            z1sb = pfix.tile([32, 4 * 128], BF16)          # relu(z1), (mt, m2)
            z1t = pfix.tile([128, 4 * 32], BF16)           # z1 transposed
            ahw = pfix.tile([8, 2 * BS], F32)
            ahrow = pfix.tile([1, 128], F32)
            awrow = pfix.tile([1, 128], F32)
            awx = pfix.tile([1, BS * L], F32)
            arow = pfix.tile([1, BS * L], F32)
            ones = pfix.tile([1, 128], F32)
            afull = pfix.tile([128, BS * L], BF16)
            w67 = pfix.tile([128, K2 * 256], BF16)         # ot6/7 cols, s<K2
            # psz lhsT needs one contiguous free dim: (ct, v, path*b) layout;
            # the xw/xh reduces write it directly
            xwhR = pfix.tile([128, 2 * NCT * 128], BF16)
            if W1_FP8:
                xwh8 = pfix.tile([128, 2 * NCT * 128], FP8)
            xpad = [pxpad.tile([128, BS * 100], BF16, tag=f"xp{ct}",
                               name=f"xp{ct}") for ct in range(NCT)]
            mdp = [pmdp.tile([128, NG], BF16, tag=f"mdp{s}", name=f"mdp{s}")
                   for s in range(K2)]

            nc.gpsimd.memset(ones[:], 1.0 / (XS * WS))

            # ---- DMA issue: f1 then x tile-halves lead both HWDGE queues
            # (x gates the ybar chain -> SE chain -> conv start); f2/w1 ride
            # the gpsimd SWDGE ring so the scalar queue stays clear for the
            # activation chain.
            nc.scalar.dma_start(out=f1sb[:], in_=f1t[:])
            XH2 = BS * 100 // 2
            for ct in range(NCT):
                nc.sync.dma_start(
                    out=xpad[ct][:, 0:XH2],
                    in_=xsp[ct * 128:(ct + 1) * 128, 0:XH2])
                nc.scalar.dma_start(
                    out=xpad[ct][:, XH2:],
                    in_=xsp[ct * 128:(ct + 1) * 128, XH2:])
            nc.sync.dma_start(out=w2sb[:], in_=w2t[:])
            nc.sync.dma_start(out=identsb[:], in_=identd[:])
            nc.gpsimd.dma_start(out=f2sb[:], in_=f2pt[:])

            # w1 pacing: prime the pw1 pool with dummies whose only reader
            # depends on ybar(4); the real w1 DMAs (allocs 4+) then carry a
            # pool-reuse semaphore wait and cannot start before ~ybar(4),
            # keeping the 8.4MB w1 stream out of the startup HBM window.
            # Scheduler-proof (the wait is attached to the DMA itself).
            w1gate = pfix.tile([128, 1], F32)
            w1dmy = [pw1.tile([128, 8 * 512], FP8 if W1_FP8 else BF16,
                              tag="w1sb", name=f"w1dmy{i}") for i in range(3)]
            for dmy in w1dmy:
                nc.gpsimd.memset(dmy[:, 0:1], 0.0)

            w1sbs = []

            def issue_w1(ct):
                w1sb = pw1.tile([128, 8 * 512], FP8 if W1_FP8 else BF16,
                                tag="w1sb", name=f"w1sb{ct}")
                nc.gpsimd.dma_start(
                    out=w1sb[:], in_=w1d[ct * 128:(ct + 1) * 128, :])
                w1sbs.append(w1sb)

            # ---- weight stream bookkeeping
            wsl_tiles = {}     # (g, s) -> tile (full or main-768)
            w8_tiles = {}      # (g, p) -> fp8 pair tile
            issue_flip = [0]

            def _weng():
                # scalar carries NO weight-stream issues at all: the list
                # scheduler hoists ready DMA issues (with sem-reuse waits)
                # ahead of not-yet-ready activations, which stalls the ys
                # sigmoid chain.  Weight stream rides sync + gpsimd only.
                n = issue_flip[0]
                issue_flip[0] += 1
                if n < 8:
                    return nc.sync
                return nc.sync if (n % 2 == 0) else nc.gpsimd

            def issue_w67():
                # ot6/7 cols for steps < K2: one contiguous host-packed bulk
                # DMA (needed from step K1; gpsimd ring, behind the w1 gate)
                nc.gpsimd.dma_start(out=w67[:], in_=w67d[:])

            def issue_wsl(g, s):
                if s < len(SCHED_BF):
                    if g == 0 and s < K2:
                        t = pwm.tile([128, 768], BF16, tag="wm",
                                     name=f"wm{g}_{s}")
                        _weng().dma_start(out=t[:], in_=wttb[s][:, 0:768])
                    else:
                        t = pw.tile([128, 1024], BF16, tag="wsl",
                                    name=f"wsl{g}_{s}")
                        _weng().dma_start(out=t[:], in_=wttb[s][:])
                    wsl_tiles[(g, s)] = t
                else:
                    p = s - len(SCHED_BF)
                    t = pw8.tile([128, 2048], FP8, tag="w8", name=f"w8{g}_{p}")
                    _weng().dma_start(out=t[:], in_=wtt8[p][:])
                    w8_tiles[(g, p)] = t

            # ---- reductions. ybar sums the whole zero-padded 100-elem row
            # (contiguous = fast DVE path; pad zeros add nothing).  xh/xw
            # write straight into the xwhR (ct, v, path*b) layout.
            def reduce_ybar(ct):
                # sum only the 8 live rows (x 10 cols, pad zeros harmless):
                # 1280 elems instead of 1600 on the startup-critical chain
                with nc.allow_low_precision(reason="bf16 activations"):
                    nc.vector.reduce_sum(
                        ybar[:, ct * BS:(ct + 1) * BS],
                        xpad[ct][:].rearrange(
                            "p (b hh ww) -> p b hh ww",
                            b=BS, hh=10, ww=10)[:, :, 1:9, 0:10],
                        axis=mybir.AxisListType.XY)

            def xwhR_path(ct, path):
                # path 0 = xw (reduce over h), path 1 = xh (reduce over w)
                o = xwhR[:].rearrange(
                    "p (ct v path b) -> p ct path b v",
                    ct=NCT, v=8, path=2, b=BS)[:, ct, path]
                with nc.allow_low_precision(reason="bf16 activations"):
                    if path == 1:
                        nc.vector.reduce_sum(
                            o,
                            xpad[ct][:].rearrange(
                                "p (b hh ww) -> p b hh ww",
                                b=BS, hh=10, ww=10)[:, :, 1:9, 0:10],
                            axis=AX)
                    else:
                        nc.vector.reduce_sum(
                            o,
                            xpad[ct][:].rearrange(
                                "p (b hh ww) -> p b ww hh",
                                b=BS, hh=10, ww=10)[:, :, 1:9, 1:9],
                            axis=AX)

            def xwh8_ct(ct):
                with nc.allow_low_precision(reason="fp8 quantize"):
                    nc.vector.tensor_copy(
                        xwh8[:, ct * 256:(ct + 1) * 256],
                        xwhR[:, ct * 256:(ct + 1) * 256])

            for ct in range(NCT):
                reduce_ybar(ct)
                if ct == 4:
                    # open the w1 gate once ybar(4) exists
                    with nc.allow_low_precision(reason="w1 gate"):
                        for dmy in w1dmy:
                            nc.gpsimd.tensor_mul(
                                w1gate[:], dmy[:, 0:1],
                                ybar[:, 4 * BS:4 * BS + 1])

            # ---- SE channel attention: psy = f1 @ ybar, t1 = relu
            psy = ps.tile([64, BS], F32, tag="psB")
            for ct in range(NCT):
                nc.tensor.matmul(
                    psy[:], f1sb[:, ct * 64:(ct + 1) * 64],
                    ybar[:, ct * BS:(ct + 1) * BS],
                    start=(ct == 0), stop=(ct == NCT - 1))
            nc.scalar.activation(t1[:], psy[:], AF.Relu)

            def psf_chunk(mq):
                pt = ps.tile([128, 4 * BS], F32, tag="psA", name=f"psf{mq}")
                for sub in range(4):
                    mt = mq * 4 + sub
                    nc.tensor.matmul(
                        pt[:, sub * BS:(sub + 1) * BS],
                        f2sb[:, mt * 128:(mt + 1) * 128], t1[:],
                        start=True, stop=True)
                nc.scalar.activation(
                    ys[:, mq * 4 * BS:(mq + 1) * 4 * BS], pt[:], AF.Sigmoid)

            psf_chunk(MQ_ORDER[0])
            psf_chunk(MQ_ORDER[1])
            mq_left = list(MQ_ORDER[2:])

            # ---- spatial attention: psz accumulation [32, 512]
            psz = ps.tile([32, 512], F32, tag="psB", name="psz")
            psz_n = [0]
            NPSZ = 32 if W1_FP8 else 64

            def psz_block(ct):
                if W1_FP8:
                    for q in range(4):
                        psz_n[0] += 1
                        nc.tensor.matmul(
                            psz[:],
                            xwh8[:, ct * 256 + q * 64:ct * 256 + (q + 1) * 64]
                            .rearrange("p (a m) -> p a m", a=2),
                            w1sbs[ct][:, q * 1024:(q + 1) * 1024]
                            .rearrange("p (a m) -> p a m", a=2),
                            start=(psz_n[0] == 1), stop=(psz_n[0] == NPSZ),
                            perf_mode=DR)
                else:
                    for v in range(8):
                        psz_n[0] += 1
                        nc.tensor.matmul(
                            psz[:],
                            xwhR[:, ct * 256 + v * 32:ct * 256 + (v + 1) * 32],
                            w1sbs[ct][:, v * 512:(v + 1) * 512],
                            start=(psz_n[0] == 1), stop=(psz_n[0] == NPSZ))

            def psz_finish():
                nc.scalar.activation(z1sb[:], psz[:], AF.Relu)
                pt = ps.tile([128, 4 * 32], BF16, tag="psA", name="ztp")
                for mt in range(4):
                    nc.tensor.transpose(
                        pt[:, mt * 32:(mt + 1) * 32],
                        z1sb[:, mt * 128:(mt + 1) * 128], identsb[:])
                nc.vector.tensor_copy(z1t[:], pt[:])
                pa = ps.tile([8, 2 * BS], F32, tag="psB", name="psa")
                for mt in range(4):
                    nc.tensor.matmul(
                        pa[:], w2sb[:, mt * 8:(mt + 1) * 8],
                        z1t[:, mt * 32:(mt + 1) * 32],
                        start=(mt == 0), stop=(mt == 3))
                nc.scalar.activation(ahw[:], pa[:], AF.Sigmoid)
                nc.scalar.dma_start(out=awrow[:], in_=ahw[:, 0:BS])
                nc.scalar.dma_start(out=ahrow[:], in_=ahw[:, BS:2 * BS])
                nc.vector.tensor_copy(
                    awx[:].rearrange("p (b h w) -> p b w h", b=BS, h=H, w=W),
                    awrow[:].rearrange("p (w b) -> p b w", w=W)
                    .broadcast_to([1, BS, W, H]))
                nc.vector.tensor_mul(
                    arow[:].rearrange("p (b h w) -> p b h w", b=BS, h=H, w=W),
                    ahrow[:].rearrange("p (h b) -> p b h", h=H)
                    .broadcast_to([1, BS, H, W]),
                    awx[:].rearrange("p (b h w) -> p b h w", b=BS, h=H, w=W))

            def psb_half(half):
                pb = ps.tile([128, NG], F32, tag="psA", name=f"psb{half}")
                nc.tensor.matmul(
                    pb[:], ones[:], arow[:, half * NG:(half + 1) * NG],
                    start=True, stop=True)
                with nc.allow_low_precision(reason="attention map bf16"):
                    nc.vector.tensor_copy(
                        afull[:, half * NG:(half + 1) * NG], pb[:])

            # ---- conv machinery
            bank_tag = {0: "cv0", 1: "cv1", 2: "cv2", 3: "cv3", 4: "cv4",
                        5: "cv5", 6: "psB", 7: "psA"}
            pscv = {}
            bank_cnt = {}

            def conv_mm(g, s, ot):
                key = (g, ot)
                if key not in pscv:
                    pscv[key] = ps.tile([128, NG], F32, tag=bank_tag[ot],
                                        name=f"pscv{g}_{ot}")
                    bank_cnt[key] = 0
                bank_cnt[key] += 1
                start = bank_cnt[key] == 1
                stop = bank_cnt[key] == PER_BANK
                pt = pscv[key]
                if s < len(SCHED_BF):
                    j, ct = SCHED_BF[s]
                    dh, dw, h0, nh, w0, nw = _tap_geom(j)
                    md = md_tiles[(g, s)]
                    if g == 0 and s < K2 and ot >= 6:
                        lhsT = w67[:, s * 256 + (ot - 6) * 128:
                                   s * 256 + (ot - 5) * 128]
                    else:
                        lhsT = wsl_tiles[(g, s)][:, ot * 128:(ot + 1) * 128]
                    nc.tensor.matmul(
                        pt[:].rearrange("p (b h w) -> p b h w", b=GB, h=H, w=W)
                        [:, :, h0:h0 + nh, w0:w0 + nw],
                        lhsT,
                        md[:].rearrange("p (b h w) -> p b h w", b=GB, h=H, w=W)
                        [:, :, h0:h0 + nh, w0:w0 + nw],
                        start=start, stop=stop)
                else:
                    p = s - len(SCHED_BF)
                    md = md_tiles[(g, s)]
                    nc.tensor.matmul(
                        pt[:],
                        w8_tiles[(g, p)][:].rearrange(
                            "p (a m) -> p a m", a=2)
                        [:, :, ot * 128:(ot + 1) * 128],
                        md[:].rearrange("p (a m) -> p a m", a=2),
                        start=start, stop=stop, perf_mode=DR)

            md_tiles = {}

            def build_md(g, s):
                if s < len(SCHED_BF):
                    j, ct = SCHED_BF[s]
                    kt = j * NCT + ct
                    dh, dw, h0, nh, w0, nw = _tap_geom(j)
                    if g == 0 and s < K2:
                        md = mdp[s]
                    else:
                        md = pmd.tile([128, NG], BF16, tag="md",
                                      name=f"md{g}_{s}")
                    nc.vector.tensor_mul(
                        md[:].rearrange("p (b h w) -> p b h w", b=GB, h=H, w=W)
                        [:, :, h0:h0 + nh, w0:w0 + nw],
                        xpad[ct][:].rearrange(
                            "p (b hh ww) -> p b hh ww", b=BS, hh=10, ww=10)
                        [:, g * GB:(g + 1) * GB,
                         dh + h0:dh + h0 + nh, dw + w0:dw + w0 + nw],
                        ys[:, kt * BS + g * GB:kt * BS + (g + 1) * GB]
                        .broadcast_to([128, GB, nh, nw]))
                else:
                    p = s - len(SCHED_BF)
                    j, q = FP8_PAIRS[p]
                    dh, dw = j // 3, j % 3
                    md = pmd.tile([128, 2 * NG], FP8, tag="md8",
                                  name=f"md8{g}_{p}")
                    for pl in range(2):
                        ct = 2 * q + pl
                        kt = j * NCT + ct
                        with nc.allow_low_precision(reason="fp8 md"):
                            nc.vector.tensor_mul(
                                md[:, pl * NG:(pl + 1) * NG]
                                .rearrange("p (b h w) -> p b h w",
                                           b=GB, h=H, w=W),
                                xpad[ct][:].rearrange(
                                    "p (b hh ww) -> p b hh ww",
                                    b=BS, hh=10, ww=10)
                                [:, g * GB:(g + 1) * GB, dh:dh + 8, dw:dw + 8],
                                ys[:, kt * BS + g * GB:kt * BS + (g + 1) * GB]
                                .broadcast_to([128, GB, H, W]))
                md_tiles[(g, s)] = md

            # ---- group 0: staged bank usage + interleaved attention
            for s in range(min(LOOKAHEAD, NSTEP)):
                issue_wsl(0, s)

            catch6 = [(i, 6) for i in range(K1)]
            catch7 = [(i, 7) for i in range(K2)]
            psz_at = {12 + 2 * c: c for c in range(NCT)}  # step -> ct block
            xw_at = {3 * c + 1: c for c in range(NCT)}    # step -> xw reduce
            xh_at = {3 * c + 2: c for c in range(NCT)}    # step -> xh reduce
            # w1 0-2 early (gated by the pool dummies); 3+ just before
            # their psz deadline so the pool-reuse waits only ever delay
            # gpsimd-queued weight tiles that are needed much later
            w1_at = {8: 0, 10: 1, 12: 2}
            w1_at.update({16 + 2 * (c - 3): c for c in range(3, NCT)})
            x8_at = ({3 * c + 3: c for c in range(NCT)} if W1_FP8 else {})
            NBODY = NSTEP - NTAIL

            for s in range(NBODY):
                if s + LOOKAHEAD < NSTEP:
                    issue_wsl(0, s + LOOKAHEAD)
                if s == 13:
                    issue_w67()
                if s in w1_at:
                    issue_w1(w1_at[s])
                if s in xw_at:
                    xwhR_path(xw_at[s], 0)
                if s in xh_at:
                    xwhR_path(xh_at[s], 1)
                if s in x8_at:
                    xwh8_ct(x8_at[s])
                build_md(0, s)
                if 1 <= s <= len(mq_left):
                    psf_chunk(mq_left[s - 1])
                if s in psz_at:
                    psz_block(psz_at[s])
                if s == K1 - 2:
                    psz_finish()
                if s == K2 - 2:
                    psb_half(0)
                if s == K2 - 1:
                    psb_half(1)
                ots = list(range(6))
                if s >= K1:
                    ots.append(6)
                if s >= K2:
                    ots.append(7)
                if s >= K1 and catch6:
                    for _ in range(2):
                        if catch6:
                            i, ot = catch6.pop(0)
                            conv_mm(0, i, ot)
                if s >= K2 and catch7:
                    for _ in range(2):
                        if catch7:
                            i, ot = catch7.pop(0)
                            conv_mm(0, i, ot)
                for ot in ots:
                    conv_mm(0, s, ot)
            assert not catch6 and not catch7

            # ---- group 0 tail (bank-major so per-bank obs overlap mms),
            #      group 1 prefetch
            for s in range(NBODY, NSTEP):
                build_md(0, s)
            for s in range(min(LOOKAHEAD, NSTEP)):
                issue_wsl(1, s)
            for s in range(3):
                build_md(1, s)

            def ob_out(g, ot):
                obt = pout.tile([128, NG], F32, tag="ob", name=f"ob{g}_{ot}")
                nc.vector.tensor_mul(
                    obt[:], pscv[(g, ot)][:],
                    afull[:, g * NG:(g + 1) * NG])
                # split the store across both HWDGE queues; the kernel's
                # final two stores get 4-way splits (they are the critical
                # tail and transfers are single-DMA-engine latency-bound)
                if g == 1 and ot >= 6:
                    engs = (nc.sync, nc.scalar, nc.sync, nc.scalar)
                    qn = NG // 4
                    for ci, eng in enumerate(engs):
                        eng.dma_start(
                            out=out[ot, :,
                                    g * NG + ci * qn:g * NG + (ci + 1) * qn],
                            in_=obt[:, ci * qn:(ci + 1) * qn])
                else:
                    e1, e2 = (nc.sync, nc.scalar) if ot % 2 == 0 else \
                        (nc.scalar, nc.sync)
                    hn = NG // 2
                    e1.dma_start(
                        out=out[ot, :, g * NG:g * NG + hn], in_=obt[:, 0:hn])
                    e2.dma_start(
                        out=out[ot, :, g * NG + hn:(g + 1) * NG],
                        in_=obt[:, hn:])

            for ot in range(NOT):
                for s in range(NBODY, NSTEP):
                    conv_mm(0, s, ot)
                ob_out(0, ot)

            # ---- group 1
            for s in range(NBODY):
                if s + LOOKAHEAD < NSTEP:
                    issue_wsl(1, s + LOOKAHEAD)
                if s >= 3:
                    build_md(1, s)
                for ot in range(NOT):
                    conv_mm(1, s, ot)
            for s in range(NBODY, NSTEP):
                build_md(1, s)
            for ot in range(NOT):
                for s in range(NBODY, NSTEP):
                    conv_mm(1, s, ot)
                ob_out(1, ot)

    _split_excess_waits(nc)
    return nc


_NC_CACHE = []


def _host_pack(x, weight, w1, w2, f1, f2):
    bf = ml_dtypes.bfloat16
    f8 = ml_dtypes.float8_e4m3
    wjt = weight.reshape(C, C, 9).transpose(2, 1, 0)       # [j, c, o]
    wttb = np.empty((NBF * NCT, 128, 1024), bf)
    for s, (j, ct) in enumerate(SCHED_BF):
        wttb[s] = (wjt[j, ct * 128:(ct + 1) * 128] * WS).astype(bf)
    # ot6/7 cols for steps < K2 packed contiguously: [128, (s, c)]
    w67d = np.ascontiguousarray(
        wttb[:K2, :, 768:1024].transpose(1, 0, 2)).reshape(128, K2 * 256)
    packs = dict(wttb=wttb, w67d=w67d)
    if FP8_PAIRS:
        wtt8 = np.empty((len(FP8_PAIRS), 128, 2, 1024), f8)
        for p, (j, q) in enumerate(FP8_PAIRS):
            for pl in range(2):
                ct = 2 * q + pl
                wtt8[p, :, pl] = np.clip(
                    wjt[j, ct * 128:(ct + 1) * 128] * WS, -240, 240).astype(f8)
        packs["wtt8"] = wtt8.reshape(len(FP8_PAIRS), 128, 2048)
    # w1: [c, v, m] layout; bf16 scale 1/64 (xwh = 64*mean), fp8 scale 256
    w1cvm = w1.reshape(512, C, 8).transpose(1, 2, 0)       # [c, v, m]
    if W1_FP8:
        packs["w1d"] = np.clip(w1cvm * 256.0, -240, 240).astype(f8).reshape(
            C, 8 * 512)
        w2scale = 1.0 / (64.0 * 256.0)
    else:
        packs["w1d"] = (w1cvm / 64.0).astype(bf).reshape(C, 8 * 512)
        w2scale = 1.0
    # ybar tile = (XS*64)*mean = 512*mean -> f1 scale 1/512
    packs["f1t"] = np.ascontiguousarray(
        (f1.T / (XS * 64.0)).reshape(8, 128, 64).transpose(1, 0, 2)
        .reshape(128, 512)).astype(bf)
    packs["f2pt"] = np.ascontiguousarray(
        f2.reshape(C, 9, 64).transpose(1, 0, 2).reshape(9216, 64).T).astype(bf)
    packs["w2t"] = np.ascontiguousarray(
        (w2.T * w2scale).reshape(4, 128, 8).transpose(1, 0, 2)
        .reshape(128, 32)).astype(bf)
    packs["identd"] = np.eye(32, dtype=np.float32).astype(bf)
    return packs


def kernel(x, weight, w1, w2, f1, f2):
    global LAST_PROFILE
    bf = ml_dtypes.bfloat16

    packs = _host_pack(np.asarray(x), np.asarray(weight), np.asarray(w1),
                       np.asarray(w2), np.asarray(f1), np.asarray(f2))

    in_maps = []
    for i in range(NCORES):
        xsh = np.asarray(x)[i * BS:(i + 1) * BS]           # [16, C, H, W]
        xspz = np.zeros((C, BS, 10, 10), bf)
        xspz[:, :, 1:9, 1:9] = (xsh.transpose(1, 0, 2, 3) * XS).astype(bf)
        m = dict(xsp=xspz.reshape(C, BS * 100))
        m.update(packs)
        in_maps.append(m)

    if not _NC_CACHE:
        _NC_CACHE.append(_build())
    nc = _NC_CACHE[0]

    kw = {}
    if TRACE:
        kw = dict(trace=True, tmpdir=TRACE_DIR)
    r = run_bass_kernel_spmd(nc, in_maps, core_ids=list(range(NCORES)), **kw)
    if TRACE:
        LAST_PROFILE = dict(
            exec_time_ns=r.exec_time_ns,
            mean_exec_time_ns=r.mean_exec_time_ns,
            profile_json=r.profile_json,
            trace=(r.instructions_and_trace[1]
                   if r.instructions_and_trace else None),
        )

    outa = np.empty((B, C, H, W), np.float32)
    for i in range(NCORES):
        res = r.results[i]["out"]                          # [8, 128, BS*L]
        outa[i * BS:(i + 1) * BS] = (
            res.reshape(NOT, 128, BS, L).transpose(2, 0, 1, 3)
            .reshape(BS, C, H, W))
    return outa


# revision 79
# speedup vs baseline: 1.0324x; 1.0324x over previous
"""Trainium2 Bass kernel for nn_ALayer_DR1_wh_light_v1 (dense_cnn).

Data-parallel over batch: 16 samples per NeuronCore, no collectives.

v3 schedule: v2 conv/attention interleave plus fp8 DoubleRow for taps
(4, 0, 8) — error measured 1.79e-2 against the 2e-2 gate, the 3-tap
subset with the best margin/savings ratio.  fp8 matmuls are w-trimmed
(boundary columns skipped; zero-padded rows contribute nothing).  The
final stores are split across both HWDGE queues.
"""
import numpy as np
import ml_dtypes

import concourse.bass as bass
import concourse.mybir as mybir
import concourse.tile as tile
from concourse.bass_utils import run_bass_kernel_spmd
from concourse.vector_clock import ScopedClock

F32 = mybir.dt.float32
BF16 = mybir.dt.bfloat16
FP8 = mybir.dt.float8e4
AX = mybir.AxisListType.X
AF = mybir.ActivationFunctionType
DR = mybir.MatmulPerfMode.DoubleRow

B, C, H, W = 128, 1024, 8, 8
L = H * W
NCORES = 8
BS = B // NCORES      # 16 samples per core
G = 2                 # sample groups
GB = BS // G          # 8 samples per group
NG = GB * L           # 512 = matmul N per group
NCT = 8               # channel tiles
NOT = 8               # output channel tiles

# ---- precision config ----------------------------------------------------
FP8_TAPS = (4, 1, 5)  # taps (j) computed in fp8 DoubleRow; () = all bf16
W1_FP8 = False        # spatial-attention w1 matmul in fp8 DoubleRow

XS = 8.0              # x prescale (host)
WS = 64.0             # conv weight prescale (host); psum scale = XS*WS

BF_TAPS = [j for j in (4, 1, 2, 3, 5, 6, 7, 0, 8) if j not in FP8_TAPS]
NBF = len(BF_TAPS)
SCHED_BF = [(j, ct) for j in BF_TAPS for ct in range(NCT)]   # bf16 steps
FP8_PAIRS = [(j, q) for j in FP8_TAPS for q in range(NCT // 2)]
NSTEP = len(SCHED_BF) + len(FP8_PAIRS)
PER_BANK = NSTEP      # matmul emissions per psum bank per group

# catch-up thresholds (tuned from traces)
K1 = 29               # step at which ot6 joins (bank freed by psz/psa)
K2 = 31               # step at which ot7 joins (bank freed by psf/psb)
NTAIL = 8             # trailing steps emitted bank-major so obs overlap

# psf chunk interleave order (mq = kt//4), deadline-ordered for SCHED_BF
MQ_ORDER = [2 * BF_TAPS[0], 2 * BF_TAPS[0] + 1]
for j in BF_TAPS[1:]:
    MQ_ORDER += [2 * j, 2 * j + 1]
for j in FP8_TAPS:
    MQ_ORDER += [2 * j, 2 * j + 1]

LOOKAHEAD = 8         # wsl DMA issue lookahead (steps)

TRACE = False
TRACE_DIR = None
LAST_PROFILE = {}

# ---------------------------------------------------------------------------
# Workaround: the staged walrus rejects instructions with >1 sync-wait
# command. Hoist excess waits onto same-engine NOPs inserted before the
# instruction (engine queues issue in order, so semantics are unchanged).
_MAXW = 1
_ctr = [0]


def _split_excess_waits(nc):
    for f in nc.m.functions:
        for blk in f.blocks:
            insts = blk.instructions
            new = []
            changed = False
            for inst in insts:
                si = inst.sync_info
                waits = list(si.on_wait) if si and si.on_wait else []
                if len(waits) > _MAXW:
                    excess = waits[:-_MAXW]
                    si.on_wait = waits[-_MAXW:]
                    while excess:
                        chunk = excess[:_MAXW]
                        excess = excess[_MAXW:]
                        _ctr[0] += 1
                        new.append(mybir.InstNoOp(
                            name=f"I-wsplit-{_ctr[0]}",
                            engine=inst.engine,
                            sync_info=mybir.SyncInfo(on_wait=chunk, on_update=[]),
                        ))
                    changed = True
                new.append(inst)
            if changed:
                insts.clear()
                insts.extend(new)


class _TileContext(tile.TileContext):
    def _drain_and_barrier(self, tick_clock, wait_clock):
        drain_inst = self.nc.sync.drain()
        wait_clock.add_sem_waits(
            drain_inst.ins, ScopedClock({None: tick_clock.global_clock})
        )
        self.nc.all_engine_barrier()
        assert self.sems is not None
        popped = self.nc._tile_sem_poison_stack.pop()
        assert popped is self._sem_poison
        self.nc.clear_and_free_semaphores(list(self.sems.allocated().values()))
        self.nc.all_engine_barrier()


# ---------------------------------------------------------------------------

def _tap_geom(j):
    dh, dw = j // 3, j % 3
    h0 = 1 if dh == 0 else 0
    nh = 8 - (1 if dh != 1 else 0)
    w0 = 1 if dw == 0 else 0
    nw = 8 - (1 if dw != 1 else 0)
    return dh, dw, h0, nh, w0, nw


def _build():
    nc = bass.Bass()
    xsp = nc.declare_dram_parameter("xsp", [C, BS * 100], BF16, isOutput=False)
    wttb = nc.declare_dram_parameter("wttb", [NBF * NCT, 128, 1024], BF16,
                                     isOutput=False)
    w1d = nc.declare_dram_parameter(
        "w1d", [C, 8 * 512], FP8 if W1_FP8 else BF16, isOutput=False)
    w67d = nc.declare_dram_parameter("w67d", [128, K2 * 256], BF16,
                                     isOutput=False)
    f1t = nc.declare_dram_parameter("f1t", [128, 8 * 64], BF16, isOutput=False)
    f2pt = nc.declare_dram_parameter("f2pt", [64, 9216], BF16, isOutput=False)
    w2t = nc.declare_dram_parameter("w2t", [128, 4 * 8], BF16, isOutput=False)
    identd = nc.declare_dram_parameter("identd", [32, 32], BF16, isOutput=False)
    if FP8_PAIRS:
        wtt8 = nc.declare_dram_parameter(
            "wtt8", [len(FP8_PAIRS), 128, 2048], FP8, isOutput=False)
    out = nc.declare_dram_parameter("out", [NOT, 128, BS * L], F32, isOutput=True)

    with _TileContext(nc) as tc:
        with (
            tc.tile_pool(name="pfix", bufs=1) as pfix,
            tc.tile_pool(name="pxpad", bufs=1) as pxpad,
            tc.tile_pool(name="pmdp", bufs=1) as pmdp,
            tc.tile_pool(name="pmd", bufs=14) as pmd,
            tc.tile_pool(name="pw", bufs=8) as pw,
            tc.tile_pool(name="pw8", bufs=4) as pw8,
            tc.tile_pool(name="pwm", bufs=8) as pwm,
            tc.tile_pool(name="pw1", bufs=3) as pw1,
            tc.tile_pool(name="pout", bufs=4) as pout,
            tc.tile_pool(name="ps", bufs=1, space="PSUM") as ps,
        ):
            # ---- persistent small tensors
            ybar = pfix.tile([128, NCT * BS], BF16)        # (ct, b)
            ys = pfix.tile([128, 9 * NCT * BS], BF16)      # (kt, b), kt=j*8+ct
            f1sb = pfix.tile([128, NCT * 64], BF16)
            f2sb = pfix.tile([64, 9216], BF16)
            w2sb = pfix.tile([128, 4 * 8], BF16)
            identsb = pfix.tile([32, 32], BF16)
            t1 = pfix.tile([64, BS], BF16)
            z1sb = pfix.tile([32, 4 * 128], BF16)          # relu(z1), (mt, m2)
            z1t = pfix.tile([128, 4 * 32], BF16)           # z1 transposed
            ahw = pfix.tile([8, 2 * BS], F32)
            ahrow = pfix.tile([1, 128], F32)
            awrow = pfix.tile([1, 128], F32)
            awx = pfix.tile([1, BS * L], F32)
            arow = pfix.tile([1, BS * L], F32)
            ones = pfix.tile([1, 128], F32)
            afull = pfix.tile([128, BS * L], BF16)
            w67 = pfix.tile([128, K2 * 256], BF16)         # ot6/7 cols, s<K2
            # psz lhsT needs one contiguous free dim: (ct, v, path*b) layout;
            # the xw/xh reduces write it directly
            xwhR = pfix.tile([128, 2 * NCT * 128], BF16)
            if W1_FP8:
                xwh8 = pfix.tile([128, 2 * NCT * 128], FP8)
            xpad = [pxpad.tile([128, BS * 100], BF16, tag=f"xp{ct}",
                               name=f"xp{ct}") for ct in range(NCT)]
            mdp = [pmdp.tile([128, NG], BF16, tag=f"mdp{s}", name=f"mdp{s}")
                   for s in range(K2)]

            nc.gpsimd.memset(ones[:], 1.0 / (XS * WS))

            # ---- DMA issue: f1 then x tile-halves lead both HWDGE queues
            # (x gates the ybar chain -> SE chain -> conv start); f2/w1 ride
            # the gpsimd SWDGE ring so the scalar queue stays clear for the
            # activation chain.
            nc.scalar.dma_start(out=f1sb[:], in_=f1t[:])
            XH2 = BS * 100 // 2
            for ct in range(NCT):
                nc.sync.dma_start(
                    out=xpad[ct][:, 0:XH2],
                    in_=xsp[ct * 128:(ct + 1) * 128, 0:XH2])
                nc.scalar.dma_start(
                    out=xpad[ct][:, XH2:],
                    in_=xsp[ct * 128:(ct + 1) * 128, XH2:])
            nc.sync.dma_start(out=w2sb[:], in_=w2t[:])
            nc.sync.dma_start(out=identsb[:], in_=identd[:])
            nc.gpsimd.dma_start(out=f2sb[:], in_=f2pt[:])

            # w1 pacing: prime the pw1 pool with dummies whose only reader
            # depends on ybar(4); the real w1 DMAs (allocs 4+) then carry a
            # pool-reuse semaphore wait and cannot start before ~ybar(4),
            # keeping the 8.4MB w1 stream out of the startup HBM window.
            # Scheduler-proof (the wait is attached to the DMA itself).
            w1gate = pfix.tile([128, 1], F32)
            w1dmy = [pw1.tile([128, 8 * 512], FP8 if W1_FP8 else BF16,
                              tag="w1sb", name=f"w1dmy{i}") for i in range(3)]
            for dmy in w1dmy:
                nc.gpsimd.memset(dmy[:, 0:1], 0.0)

            w1sbs = []

            def issue_w1(ct):
                w1sb = pw1.tile([128, 8 * 512], FP8 if W1_FP8 else BF16,
                                tag="w1sb", name=f"w1sb{ct}")
                nc.gpsimd.dma_start(
                    out=w1sb[:], in_=w1d[ct * 128:(ct + 1) * 128, :])
                w1sbs.append(w1sb)

            # ---- weight stream bookkeeping
            wsl_tiles = {}     # (g, s) -> tile (full or main-768)
            w8_tiles = {}      # (g, p) -> fp8 pair tile
            issue_flip = [0]

            def _weng():
                # scalar carries NO weight-stream issues at all: the list
                # scheduler hoists ready DMA issues (with sem-reuse waits)
                # ahead of not-yet-ready activations, which stalls the ys
                # sigmoid chain.  Weight stream rides sync + gpsimd only.
                n = issue_flip[0]
                issue_flip[0] += 1
                if n < 8:
                    return nc.sync
                return nc.sync if (n % 2 == 0) else nc.gpsimd

            def issue_w67():
                # ot6/7 cols for steps < K2: one contiguous host-packed bulk
                # DMA (needed from step K1; gpsimd ring, behind the w1 gate)
                nc.gpsimd.dma_start(out=w67[:], in_=w67d[:])

            def issue_wsl(g, s):
                if s < len(SCHED_BF):
                    if g == 0 and s < K2:
                        t = pwm.tile([128, 768], BF16, tag="wm",
                                     name=f"wm{g}_{s}")
                        _weng().dma_start(out=t[:], in_=wttb[s][:, 0:768])
                    else:
                        t = pw.tile([128, 1024], BF16, tag="wsl",
                                    name=f"wsl{g}_{s}")
                        _weng().dma_start(out=t[:], in_=wttb[s][:])
                    wsl_tiles[(g, s)] = t
                else:
                    p = s - len(SCHED_BF)
                    t = pw8.tile([128, 2048], FP8, tag="w8", name=f"w8{g}_{p}")
                    _weng().dma_start(out=t[:], in_=wtt8[p][:])
                    w8_tiles[(g, p)] = t

            # ---- reductions. ybar sums the whole zero-padded 100-elem row
            # (contiguous = fast DVE path; pad zeros add nothing).  xh/xw
            # write straight into the xwhR (ct, v, path*b) layout.
            def reduce_ybar(ct):
                # sum only the 8 live rows (x 10 cols, pad zeros harmless):
                # 1280 elems instead of 1600 on the startup-critical chain
                with nc.allow_low_precision(reason="bf16 activations"):
                    nc.vector.reduce_sum(
                        ybar[:, ct * BS:(ct + 1) * BS],
                        xpad[ct][:].rearrange(
                            "p (b hh ww) -> p b hh ww",
                            b=BS, hh=10, ww=10)[:, :, 1:9, 0:10],
                        axis=mybir.AxisListType.XY)

            def xwhR_path(ct, path):
                # path 0 = xw (reduce over h), path 1 = xh (reduce over w)
                o = xwhR[:].rearrange(
                    "p (ct v path b) -> p ct path b v",
                    ct=NCT, v=8, path=2, b=BS)[:, ct, path]
                with nc.allow_low_precision(reason="bf16 activations"):
                    if path == 1:
                        nc.vector.reduce_sum(
                            o,
                            xpad[ct][:].rearrange(
                                "p (b hh ww) -> p b hh ww",
                                b=BS, hh=10, ww=10)[:, :, 1:9, 0:10],
                            axis=AX)
                    else:
                        nc.vector.reduce_sum(
                            o,
                            xpad[ct][:].rearrange(
                                "p (b hh ww) -> p b ww hh",
                                b=BS, hh=10, ww=10)[:, :, 1:9, 1:9],
                            axis=AX)

            def xwh8_ct(ct):
                with nc.allow_low_precision(reason="fp8 quantize"):
                    nc.vector.tensor_copy(
                        xwh8[:, ct * 256:(ct + 1) * 256],
                        xwhR[:, ct * 256:(ct + 1) * 256])

            for ct in range(NCT):
                reduce_ybar(ct)
                if ct == 4:
                    # open the w1 gate once ybar(4) exists
                    with nc.allow_low_precision(reason="w1 gate"):
                        for dmy in w1dmy:
                            nc.gpsimd.tensor_mul(
                                w1gate[:], dmy[:, 0:1],
                                ybar[:, 4 * BS:4 * BS + 1])

            # ---- SE channel attention: psy = f1 @ ybar, t1 = relu
            psy = ps.tile([64, BS], F32, tag="psB")
            for ct in range(NCT):
                nc.tensor.matmul(
                    psy[:], f1sb[:, ct * 64:(ct + 1) * 64],
                    ybar[:, ct * BS:(ct + 1) * BS],
                    start=(ct == 0), stop=(ct == NCT - 1))
            nc.scalar.activation(t1[:], psy[:], AF.Relu)

            def psf_chunk(mq):
                pt = ps.tile([128, 4 * BS], F32, tag="psA", name=f"psf{mq}")
                for sub in range(4):
                    mt = mq * 4 + sub
                    nc.tensor.matmul(
                        pt[:, sub * BS:(sub + 1) * BS],
                        f2sb[:, mt * 128:(mt + 1) * 128], t1[:],
                        start=True, stop=True)
                nc.scalar.activation(
                    ys[:, mq * 4 * BS:(mq + 1) * 4 * BS], pt[:], AF.Sigmoid)

            psf_chunk(MQ_ORDER[0])
            psf_chunk(MQ_ORDER[1])
            mq_left = list(MQ_ORDER[2:])

            # ---- spatial attention: psz accumulation [32, 512]
            psz = ps.tile([32, 512], F32, tag="psB", name="psz")
            psz_n = [0]
            NPSZ = 32 if W1_FP8 else 64

            def psz_block(ct):
                if W1_FP8:
                    for q in range(4):
                        psz_n[0] += 1
                        nc.tensor.matmul(
                            psz[:],
                            xwh8[:, ct * 256 + q * 64:ct * 256 + (q + 1) * 64]
                            .rearrange("p (a m) -> p a m", a=2),
                            w1sbs[ct][:, q * 1024:(q + 1) * 1024]
                            .rearrange("p (a m) -> p a m", a=2),
                            start=(psz_n[0] == 1), stop=(psz_n[0] == NPSZ),
                            perf_mode=DR)
                else:
                    for v in range(8):
                        psz_n[0] += 1
                        nc.tensor.matmul(
                            psz[:],
                            xwhR[:, ct * 256 + v * 32:ct * 256 + (v + 1) * 32],
                            w1sbs[ct][:, v * 512:(v + 1) * 512],
                            start=(psz_n[0] == 1), stop=(psz_n[0] == NPSZ))

            def psz_finish():
                nc.scalar.activation(z1sb[:], psz[:], AF.Relu)
                pt = ps.tile([128, 4 * 32], BF16, tag="psA", name="ztp")
                for mt in range(4):
                    nc.tensor.transpose(
                        pt[:, mt * 32:(mt + 1) * 32],
                        z1sb[:, mt * 128:(mt + 1) * 128], identsb[:])
                nc.vector.tensor_copy(z1t[:], pt[:])
                pa = ps.tile([8, 2 * BS], F32, tag="psB", name="psa")
                for mt in range(4):
                    nc.tensor.matmul(
                        pa[:], w2sb[:, mt * 8:(mt + 1) * 8],
                        z1t[:, mt * 32:(mt + 1) * 32],
                        start=(mt == 0), stop=(mt == 3))
                nc.scalar.activation(ahw[:], pa[:], AF.Sigmoid)
                nc.scalar.dma_start(out=awrow[:], in_=ahw[:, 0:BS])
                nc.scalar.dma_start(out=ahrow[:], in_=ahw[:, BS:2 * BS])
                nc.vector.tensor_copy(
                    awx[:].rearrange("p (b h w) -> p b w h", b=BS, h=H, w=W),
                    awrow[:].rearrange("p (w b) -> p b w", w=W)
                    .broadcast_to([1, BS, W, H]))
                nc.vector.tensor_mul(
                    arow[:].rearrange("p (b h w) -> p b h w", b=BS, h=H, w=W),
                    ahrow[:].rearrange("p (h b) -> p b h", h=H)
                    .broadcast_to([1, BS, H, W]),
                    awx[:].rearrange("p (b h w) -> p b h w", b=BS, h=H, w=W))

            def psb_half(half):
                pb = ps.tile([128, NG], F32, tag="psA", name=f"psb{half}")
                nc.tensor.matmul(
                    pb[:], ones[:], arow[:, half * NG:(half + 1) * NG],
                    start=True, stop=True)
                with nc.allow_low_precision(reason="attention map bf16"):
                    nc.vector.tensor_copy(
                        afull[:, half * NG:(half + 1) * NG], pb[:])

            # ---- conv machinery
            bank_tag = {0: "cv0", 1: "cv1", 2: "cv2", 3: "cv3", 4: "cv4",
                        5: "cv5", 6: "psB", 7: "psA"}
            pscv = {}
            bank_cnt = {}

            def conv_mm(g, s, ot):
                key = (g, ot)
                if key not in pscv:
                    pscv[key] = ps.tile([128, NG], F32, tag=bank_tag[ot],
                                        name=f"pscv{g}_{ot}")
                    bank_cnt[key] = 0
                bank_cnt[key] += 1
                start = bank_cnt[key] == 1
                stop = bank_cnt[key] == PER_BANK
                pt = pscv[key]
                if s < len(SCHED_BF):
                    j, ct = SCHED_BF[s]
                    dh, dw, h0, nh, w0, nw = _tap_geom(j)
                    md = md_tiles[(g, s)]
                    if g == 0 and s < K2 and ot >= 6:
                        lhsT = w67[:, s * 256 + (ot - 6) * 128:
                                   s * 256 + (ot - 5) * 128]
                    else:
                        lhsT = wsl_tiles[(g, s)][:, ot * 128:(ot + 1) * 128]
                    nc.tensor.matmul(
                        pt[:].rearrange("p (b h w) -> p b h w", b=GB, h=H, w=W)
                        [:, :, h0:h0 + nh, w0:w0 + nw],
                        lhsT,
                        md[:].rearrange("p (b h w) -> p b h w", b=GB, h=H, w=W)
                        [:, :, h0:h0 + nh, w0:w0 + nw],
                        start=start, stop=stop)
                else:
                    p = s - len(SCHED_BF)
                    md = md_tiles[(g, s)]
                    nc.tensor.matmul(
                        pt[:],
                        w8_tiles[(g, p)][:].rearrange(
                            "p (a m) -> p a m", a=2)
                        [:, :, ot * 128:(ot + 1) * 128],
                        md[:].rearrange("p (a m) -> p a m", a=2),
                        start=start, stop=stop, perf_mode=DR)

            md_tiles = {}

            def build_md(g, s):
                if s < len(SCHED_BF):
                    j, ct = SCHED_BF[s]
                    kt = j * NCT + ct
                    dh, dw, h0, nh, w0, nw = _tap_geom(j)
                    if g == 0 and s < K2:
                        md = mdp[s]
                    else:
                        md = pmd.tile([128, NG], BF16, tag="md",
                                      name=f"md{g}_{s}")
                    nc.vector.tensor_mul(
                        md[:].rearrange("p (b h w) -> p b h w", b=GB, h=H, w=W)
                        [:, :, h0:h0 + nh, w0:w0 + nw],
                        xpad[ct][:].rearrange(
                            "p (b hh ww) -> p b hh ww", b=BS, hh=10, ww=10)
                        [:, g * GB:(g + 1) * GB,
                         dh + h0:dh + h0 + nh, dw + w0:dw + w0 + nw],
                        ys[:, kt * BS + g * GB:kt * BS + (g + 1) * GB]
                        .broadcast_to([128, GB, nh, nw]))
                else:
                    p = s - len(SCHED_BF)
                    j, q = FP8_PAIRS[p]
                    dh, dw = j // 3, j % 3
                    md = pmd.tile([128, 2 * NG], FP8, tag="md8",
                                  name=f"md8{g}_{p}")
                    for pl in range(2):
                        ct = 2 * q + pl
                        kt = j * NCT + ct
                        with nc.allow_low_precision(reason="fp8 md"):
                            nc.vector.tensor_mul(
                                md[:, pl * NG:(pl + 1) * NG]
                                .rearrange("p (b h w) -> p b h w",
                                           b=GB, h=H, w=W),
                                xpad[ct][:].rearrange(
                                    "p (b hh ww) -> p b hh ww",
                                    b=BS, hh=10, ww=10)
                                [:, g * GB:(g + 1) * GB, dh:dh + 8, dw:dw + 8],
                                ys[:, kt * BS + g * GB:kt * BS + (g + 1) * GB]
                                .broadcast_to([128, GB, H, W]))
                md_tiles[(g, s)] = md

            # ---- group 0: staged bank usage + interleaved attention
            for s in range(min(LOOKAHEAD, NSTEP)):
                issue_wsl(0, s)

            catch6 = [(i, 6) for i in range(K1)]
            catch7 = [(i, 7) for i in range(K2)]
            psz_at = {12 + 2 * c: c for c in range(NCT)}  # step -> ct block
            xw_at = {3 * c + 1: c for c in range(NCT)}    # step -> xw reduce
            xh_at = {3 * c + 2: c for c in range(NCT)}    # step -> xh reduce
            # w1 0-2 early (gated by the pool dummies); 3+ just before
            # their psz deadline so the pool-reuse waits only ever delay
            # gpsimd-queued weight tiles that are needed much later
            w1_at = {8: 0, 10: 1, 12: 2}
            w1_at.update({16 + 2 * (c - 3): c for c in range(3, NCT)})
            x8_at = ({3 * c + 3: c for c in range(NCT)} if W1_FP8 else {})
            NBODY = NSTEP - NTAIL

            for s in range(NBODY):
                if s + LOOKAHEAD < NSTEP:
                    issue_wsl(0, s + LOOKAHEAD)
                if s == 13:
                    issue_w67()
                if s in w1_at:
                    issue_w1(w1_at[s])
                if s in xw_at:
                    xwhR_path(xw_at[s], 0)
                if s in xh_at:
                    xwhR_path(xh_at[s], 1)
                if s in x8_at:
                    xwh8_ct(x8_at[s])
                build_md(0, s)
                if 1 <= s <= len(mq_left):
                    psf_chunk(mq_left[s - 1])
                if s in psz_at:
                    psz_block(psz_at[s])
                if s == K1 - 2:
                    psz_finish()
                if s == K2 - 2:
                    psb_half(0)
                if s == K2 - 1:
                    psb_half(1)
                ots = list(range(6))
                if s >= K1:
                    ots.append(6)
                if s >= K2:
                    ots.append(7)
                if s >= K1 and catch6:
                    for _ in range(2):
                        if catch6:
                            i, ot = catch6.pop(0)
                            conv_mm(0, i, ot)
                if s >= K2 and catch7:
                    for _ in range(2):
                        if catch7:
                            i, ot = catch7.pop(0)
                            conv_mm(0, i, ot)
                for ot in ots:
                    conv_mm(0, s, ot)
            assert not catch6 and not catch7

            # ---- group 0 tail (bank-major so per-bank obs overlap mms),
            #      group 1 prefetch
            for s in range(NBODY, NSTEP):
                build_md(0, s)
            for s in range(min(LOOKAHEAD, NSTEP)):
                issue_wsl(1, s)
            for s in range(3):
                build_md(1, s)

            def ob_out(g, ot):
                obt = pout.tile([128, NG], F32, tag="ob", name=f"ob{g}_{ot}")
                nc.vector.tensor_mul(
                    obt[:], pscv[(g, ot)][:],
                    afull[:, g * NG:(g + 1) * NG])
                # group-0 stores ride the idle gpsimd ring so they don't
                # contend with group-1's weight stream on sync/scalar
                # (stores are latency-tolerant: ~100us before the drain);
                # the kernel's final two stores get 4-way fast-ring splits
                if g == 0:
                    nc.gpsimd.dma_start(
                        out=out[ot, :, g * NG:(g + 1) * NG], in_=obt[:])
                elif g == 1 and ot >= 6:
                    engs = (nc.sync, nc.scalar, nc.sync, nc.scalar)
                    qn = NG // 4
                    for ci, eng in enumerate(engs):
                        eng.dma_start(
                            out=out[ot, :,
                                    g * NG + ci * qn:g * NG + (ci + 1) * qn],
                            in_=obt[:, ci * qn:(ci + 1) * qn])
                else:
                    e1, e2 = (nc.sync, nc.scalar) if ot % 2 == 0 else \
                        (nc.scalar, nc.sync)
                    hn = NG // 2
                    e1.dma_start(
                        out=out[ot, :, g * NG:g * NG + hn], in_=obt[:, 0:hn])
                    e2.dma_start(
                        out=out[ot, :, g * NG + hn:(g + 1) * NG],
                        in_=obt[:, hn:])

            for ot in range(NOT):
                for s in range(NBODY, NSTEP):
                    conv_mm(0, s, ot)
                ob_out(0, ot)

            # ---- group 1
            for s in range(NBODY):
                if s + LOOKAHEAD < NSTEP:
                    issue_wsl(1, s + LOOKAHEAD)
                if s >= 3:
                    build_md(1, s)
                for ot in range(NOT):
                    conv_mm(1, s, ot)
            for s in range(NBODY, NSTEP):
                build_md(1, s)
            for ot in range(NOT):
                for s in range(NBODY, NSTEP):
                    conv_mm(1, s, ot)
                ob_out(1, ot)

    _split_excess_waits(nc)
    return nc


_NC_CACHE = []


def _host_pack(x, weight, w1, w2, f1, f2):
    bf = ml_dtypes.bfloat16
    f8 = ml_dtypes.float8_e4m3
    wjt = weight.reshape(C, C, 9).transpose(2, 1, 0)       # [j, c, o]
    wttb = np.empty((NBF * NCT, 128, 1024), bf)
    for s, (j, ct) in enumerate(SCHED_BF):
        wttb[s] = (wjt[j, ct * 128:(ct + 1) * 128] * WS).astype(bf)
    # ot6/7 cols for steps < K2 packed contiguously: [128, (s, c)]
    w67d = np.ascontiguousarray(
        wttb[:K2, :, 768:1024].transpose(1, 0, 2)).reshape(128, K2 * 256)
    packs = dict(wttb=wttb, w67d=w67d)
    if FP8_PAIRS:
        wtt8 = np.empty((len(FP8_PAIRS), 128, 2, 1024), f8)
        for p, (j, q) in enumerate(FP8_PAIRS):
            for pl in range(2):
                ct = 2 * q + pl
                wtt8[p, :, pl] = np.clip(
                    wjt[j, ct * 128:(ct + 1) * 128] * WS, -240, 240).astype(f8)
        packs["wtt8"] = wtt8.reshape(len(FP8_PAIRS), 128, 2048)
    # w1: [c, v, m] layout; bf16 scale 1/64 (xwh = 64*mean), fp8 scale 256
    w1cvm = w1.reshape(512, C, 8).transpose(1, 2, 0)       # [c, v, m]
    if W1_FP8:
        packs["w1d"] = np.clip(w1cvm * 256.0, -240, 240).astype(f8).reshape(
            C, 8 * 512)
        w2scale = 1.0 / (64.0 * 256.0)
    else:
        packs["w1d"] = (w1cvm / 64.0).astype(bf).reshape(C, 8 * 512)
        w2scale = 1.0
    # ybar tile = (XS*64)*mean = 512*mean -> f1 scale 1/512
    packs["f1t"] = np.ascontiguousarray(
        (f1.T / (XS * 64.0)).reshape(8, 128, 64).transpose(1, 0, 2)
        .reshape(128, 512)).astype(bf)
    packs["f2pt"] = np.ascontiguousarray(
        f2.reshape(C, 9, 64).transpose(1, 0, 2).reshape(9216, 64).T).astype(bf)
    packs["w2t"] = np.ascontiguousarray(
        (w2.T * w2scale).reshape(4, 128, 8).transpose(1, 0, 2)
        .reshape(128, 32)).astype(bf)
    packs["identd"] = np.eye(32, dtype=np.float32).astype(bf)
    return packs


def kernel(x, weight, w1, w2, f1, f2):
    global LAST_PROFILE
    bf = ml_dtypes.bfloat16

    packs = _host_pack(np.asarray(x), np.asarray(weight), np.asarray(w1),
                       np.asarray(w2), np.asarray(f1), np.asarray(f2))

    in_maps = []
    for i in range(NCORES):
        xsh = np.asarray(x)[i * BS:(i + 1) * BS]           # [16, C, H, W]
        xspz = np.zeros((C, BS, 10, 10), bf)
        xspz[:, :, 1:9, 1:9] = (xsh.transpose(1, 0, 2, 3) * XS).astype(bf)
        m = dict(xsp=xspz.reshape(C, BS * 100))
        m.update(packs)
        in_maps.append(m)

    if not _NC_CACHE:
        _NC_CACHE.append(_build())
    nc = _NC_CACHE[0]

    kw = {}
    if TRACE:
        kw = dict(trace=True, tmpdir=TRACE_DIR)
    r = run_bass_kernel_spmd(nc, in_maps, core_ids=list(range(NCORES)), **kw)
    if TRACE:
        LAST_PROFILE = dict(
            exec_time_ns=r.exec_time_ns,
            mean_exec_time_ns=r.mean_exec_time_ns,
            profile_json=r.profile_json,
            trace=(r.instructions_and_trace[1]
                   if r.instructions_and_trace else None),
        )

    outa = np.empty((B, C, H, W), np.float32)
    for i in range(NCORES):
        res = r.results[i]["out"]                          # [8, 128, BS*L]
        outa[i * BS:(i + 1) * BS] = (
            res.reshape(NOT, 128, BS, L).transpose(2, 0, 1, 3)
            .reshape(BS, C, H, W))
    return outa
